# revision 49
# baseline (speedup 1.0000x reference)
"""Paged GQA decode attention (1 token/seq) on 8 trn2 NeuronCores.

Sharding: tensor-parallel over the 8 KV heads. Core i owns KV head i and
its G=4 query heads: Wq/Wk/Wv column-sharded, Wo row-sharded; each core
produces a partial [BS, HID] output and the host sums the 8 partials.

v7 (from v6 = all-bf16 streaming): the K half of the cache stream is
fp8 e3m4 (4-bit mantissa; ~1.5% output err vs the 2e-2 gate, halves K
bytes AND doubles the S-matmul LDWEIGHTS rate via fp8 FWL); V stays
bf16 (K+V both e3m4 measured 2.2% err - over gate). kp/vp are separate
DRAM tensors so each group issues a K DMA (needed first) then a V DMA.
Wo gets its own SBUF slot: its mid-stream DMA no longer waits on
phase A freeing wq's slot, so it can't head-of-line block the KV
stream on the SP HWDGE ring. PV matmuls are col-tiled 4x: each seq's
blocks deal round-robin onto array col-strips 0/32/64/96 (M=4 each,
concurrent streams via separate XBUSes), partials combined on ACT+DVE;
this ~3x's PV throughput so per-group PE work stays under per-group
DMA time and the stream never stalls on buffer-free semaphores.

Per-core dataflow (one Bass program, SPMD over cores via per-core inputs):
  A) kv = hidT.T @ Wkv then q = hidT.T @ Wq (PE, chunked weight DMAs);
     RMSNorm + RoPE on [BS, D] tiles (ACT/DVE), attn scale folded into
     q's inverse-rms; k path first so the cache patches unblock early.
  B) per group: K DMA (fp8) + V DMA (bf16); stale cache slot at
     pos=T-1 gets its KT column (DVE, fp8) / V row (SWDGE DMA)
     overwritten with this step's k/v; S^T[tok, 4*nbg] per block
     (lhsT=K fp8 block, rhs=qT_b bf16); exp per 16-block sub-chunk;
     per-seq tail mask; per-seq 4-strip PV with the V ones-column
     giving the softmax denominator.
  C) partial = attn^T.T @ Wo_chunk (PE), 4 chunks of 1024 cols, bf16 out.
"""

import numpy as np
import ml_dtypes

BF16 = ml_dtypes.bfloat16
F8E3 = ml_dtypes.float8_e3m4

HID, H, HKV, D = 4096, 32, 8, 128
BS, BLOCKS_PER_SEQ, BLOCK_SIZE = 32, 32, 128
G = H // HKV
EPS = 1e-6
NCORES = 8
KTILES = HID // 128  # contraction tiles for the projections
VW = D + 1  # V row width: 128 values + ones col (softmax denominator)
GMAX = 64  # max cache blocks per streamed group
EXPB = 32  # blocks per exp sub-chunk (pipelines exp/PV under the S matmuls)
WCHUNK = 8  # k-tiles per weight DMA chunk (phase A starts on first chunk)
OCH = 1024  # output columns per phase-C chunk
NSTRIP = 2  # PE col-strips for the PV matmuls

_prog_cache = {}


def _plan(nb):
    """Pack whole sequences into groups of <= GMAX blocks.

    One ~8-block sequence opens the stream (its compute finishes fast,
    releasing the first stream buffer early) and one ~12-block sequence
    closes it (short post-stream tail). The rest go first-fit-
    decreasing; bins stream smallest-first so early buffer releases
    come quickly while the PE is still catching up on phase A.
    Returns (pack_order, groups); groups are (start_block, [seq ids])."""
    idx = sorted(range(len(nb)), key=lambda b: -nb[b])
    front = min(idx, key=lambda b: abs(nb[b] - 8))
    # a deliberately tiny closing group (3 smallest seqs) keeps the
    # post-stream compute tail short
    smallest = sorted((b for b in idx if b != front), key=lambda b: nb[b])[:3]
    rest = [b for b in idx if b != front and b not in smallest]
    # LPT-balance the rest: each group mixes one big seq with several
    # small ones, so the per-seq normalize chains (fixed ACT/DVE cost)
    # spread evenly across the stream instead of bunching at the end
    ngrp = max(1, -(-sum(nb[b] for b in rest) // GMAX))
    bins = [[0, []] for _ in range(ngrp)]
    for b in rest:  # already descending
        cand = [x for x in bins if x[0] + nb[b] <= GMAX]
        if not cand:
            bins.append([0, []])
            cand = [bins[-1]]
        bin_ = min(cand, key=lambda x: x[0])
        bin_[0] += nb[b]
        bin_[1].append(b)
    bins.sort(key=lambda x: -x[0])
    bins = ([[nb[front], [front]]] + bins
            + [[sum(nb[b] for b in smallest), smallest]])
    groups, order, start = [], [], 0
    for tot, seqs in bins:
        groups.append((start, seqs))
        order.extend(seqs)
        start += tot
    return order, groups


def _build_program(seq_lens, apply_qw, apply_kw):
    import concourse.bass as bass
    import concourse.tile as tile
    from concourse import bacc, mybir

    f32 = mybir.dt.float32
    bf16 = mybir.dt.bfloat16
    f8e3 = mybir.dt.float8e3
    AF = mybir.ActivationFunctionType

    nb = [(int(t) + BLOCK_SIZE - 1) // BLOCK_SIZE for t in seq_lens]
    nbtot = sum(nb)
    _, groups = _plan(nb)

    nc = bacc.Bacc("TRN2", target_bir_lowering=False)
    hidT = nc.dram_tensor("hidT", [128, KTILES * BS], bf16, kind="ExternalInput")
    wq = nc.dram_tensor("wq", [128, KTILES * G * D], bf16, kind="ExternalInput")
    wkv = nc.dram_tensor("wkv", [128, KTILES * 2 * D], bf16, kind="ExternalInput")
    wo = nc.dram_tensor("wo", [128, G * HID], bf16, kind="ExternalInput")
    cssn = nc.dram_tensor("cssn", [BS, 2 * D + BS], f32, kind="ExternalInput")
    kp = nc.dram_tensor("kp", [128, nbtot * BLOCK_SIZE], f8e3, kind="ExternalInput")
    vp = nc.dram_tensor("vp", [128, nbtot * VW], bf16, kind="ExternalInput")
    if apply_qw:
        qw = nc.dram_tensor("qw", [1, D], f32, kind="ExternalInput")
    if apply_kw:
        kw = nc.dram_tensor("kw", [1, D], f32, kind="ExternalInput")
    outp = nc.dram_tensor("outp", [BS, HID], bf16, kind="ExternalOutput")

    with tile.TileContext(nc) as tc:
        with tc.tile_pool(name="sb", bufs=1) as sb, tc.tile_pool(
            name="smalls", bufs=4
        ) as smalls:
            # SP-ring DMAs first: q path gates every S matmul, while the
            # k/v path only gates the per-seq cache patches. wq/wkv in
            # chunks so the projection matmuls start on the first.
            hid_sb = sb.tile([128, KTILES * BS], bf16, name="hid_sb")
            nc.sync.dma_start(out=hid_sb, in_=hidT[:, :])
            wq_sb = sb.tile([128, KTILES * G * D], bf16, name="wq_sb", tag="bigw")
            for c0 in range(0, KTILES, WCHUNK):
                nc.sync.dma_start(
                    out=wq_sb[:, c0 * G * D : (c0 + WCHUNK) * G * D],
                    in_=wq[:, c0 * G * D : (c0 + WCHUNK) * G * D],
                )
            wkv_sb = sb.tile([128, KTILES * 2 * D], bf16, name="wkv_sb")
            for c0 in range(0, KTILES, WCHUNK):
                nc.sync.dma_start(
                    out=wkv_sb[:, c0 * 2 * D : (c0 + WCHUNK) * 2 * D],
                    in_=wkv[:, c0 * 2 * D : (c0 + WCHUNK) * 2 * D],
                )
            # No on-device tail masking: the host zeroes the invalid V
            # tail rows AND their ones-column entries, so garbage exp
            # values multiply zeros and the denominator stays exact.
            # ACT-ring DMA: cos/sin + transpose identity; the streams
            # stay on the SP ring — streaming DMAs on the ACT ring
            # serialize with exp (engine head-of-line blocking)
            cssn_sb = sb.tile([BS, 2 * D + BS], f32, name="cssn_sb")
            nc.scalar.dma_start(out=cssn_sb, in_=cssn[:, :])
            cos_sb = cssn_sb[:, 0:D]
            sin_sb = cssn_sb[:, D : 2 * D]
            ident = cssn_sb[:, 2 * D : 2 * D + BS]

            norm_w_sb = {}
            for flag, name, dram in (
                (apply_qw, "qw_sb", qw if apply_qw else None),
                (apply_kw, "kw_sb", kw if apply_kw else None),
            ):
                if flag:
                    t = sb.tile([BS, D], f32, name=name)
                    src = dram[:, :]
                    bcast = bass.AP(
                        tensor=src.tensor,
                        offset=src.offset,
                        ap=[[0, BS], list(src.ap[-1])],
                    )
                    nc.sync.dma_start(out=t, in_=bcast)
                    norm_w_sb[name] = t

            eps_q = sb.tile([BS, 1], f32, name="eps_q")
            nc.vector.memset(eps_q, float(D) * EPS)
            eps_k = sb.tile([BS, 1], f32, name="eps_k")
            nc.vector.memset(eps_k, EPS)

            qr_sb = sb.tile([BS, G * D], f32, name="qr_sb")
            kr_sb = sb.tile([BS, D], f32, name="kr_sb")
            vbf = sb.tile([BS, D], bf16, name="vbf")
            qT_sb = sb.tile([128, G * BS], bf16, name="qT_sb")
            kT8 = sb.tile([128, BS], f8e3, name="kT8")
            attn_T = sb.tile([128, G * BS], bf16, name="attn_T")

            with tc.tile_pool(name="psA", bufs=1, space="PSUM") as psA:
                q_ps = psA.tile([BS, G * D], f32, name="q_ps")
                kv_ps = psA.tile([BS, 2 * D], f32, name="kv_ps")
                last = KTILES - 1
                # all q matmuls first (need only wq, which streams first)
                for t in range(KTILES):
                    nc.tensor.matmul(
                        q_ps,
                        hid_sb[:, t * BS : (t + 1) * BS],
                        wq_sb[:, t * G * D : (t + 1) * G * D],
                        start=(t == 0), stop=(t == last),
                    )
                for t in range(KTILES):
                    nc.tensor.matmul(
                        kv_ps,
                        hid_sb[:, t * BS : (t + 1) * BS],
                        wkv_sb[:, t * 2 * D : (t + 1) * 2 * D],
                        start=(t == 0), stop=(t == last),
                    )
                # vbf needs only the kv matmuls — copy it out before the
                # q norm so the per-seq V-row patches unblock early
                nc.vector.tensor_copy(vbf, kv_ps[:, D : 2 * D])
                k_ps = kv_ps[:, 0:D]
                v_ps = kv_ps[:, D : 2 * D]

                MUL = mybir.AluOpType.mult

                def norm_rope(src_ps, dst, head_cnt, is_q):
                    # ACT: Square+accum then Rsqrt (rms inverse in one
                    # table op); DVE: fused (x*inv)*cos / (xswap*inv)*sin
                    # via scalar_tensor_tensor + one add. The host
                    # NEGATES the first half of sin so both rope halves
                    # are adds.
                    w_sb = norm_w_sb.get("qw_sb" if is_q else "kw_sb")
                    for h in range(head_cnt):
                        xin = src_ps[:, h * D : (h + 1) * D]
                        scratch = smalls.tile([BS, D], f32, name="scratch", tag="scr")
                        ssq = smalls.tile([BS, 1], f32, name="ssq", tag="ssq")
                        nc.scalar.activation(scratch, xin, AF.Square, accum_out=ssq)
                        s = smalls.tile([BS, 1], f32, name="s", tag="s")
                        if is_q:
                            # s = sqrt(sum(q^2) + D*eps): 1/s folds the
                            # attention scale D**-0.5 into the rms norm
                            nc.scalar.activation(s, ssq, AF.Sqrt, bias=eps_q, scale=1.0)
                        else:
                            nc.scalar.activation(s, ssq, AF.Sqrt, bias=eps_k, scale=1.0 / D)
                        inv = smalls.tile([BS, 1], f32, name="inv", tag="inv")
                        nc.vector.reciprocal(inv, s)
                        xs = xin
                        if w_sb is not None:
                            xs = smalls.tile([BS, D], f32, name="xn", tag="xn")
                            nc.vector.tensor_mul(xs, xin, w_sb)
                        t1 = smalls.tile([BS, D], f32, name="t1", tag="t1")
                        m2 = smalls.tile([BS, D], f32, name="m2", tag="m2")
                        nc.vector.scalar_tensor_tensor(
                            t1, xs, inv, cos_sb, op0=MUL, op1=MUL
                        )
                        nc.vector.scalar_tensor_tensor(
                            m2[:, 0 : D // 2], xs[:, D // 2 : D], inv,
                            sin_sb[:, 0 : D // 2], op0=MUL, op1=MUL
                        )
                        nc.vector.scalar_tensor_tensor(
                            m2[:, D // 2 : D], xs[:, 0 : D // 2], inv,
                            sin_sb[:, D // 2 : D], op0=MUL, op1=MUL
                        )
                        nc.vector.tensor_add(dst[:, h * D : (h + 1) * D], t1, m2)

                # q norm/rope/transpose first: qT gates all S matmuls
                norm_rope(q_ps, qr_sb, G, True)
                with tc.tile_pool(name="psT", bufs=2, space="PSUM") as psT:
                    for h in range(G):
                        tp = psT.tile([128, BS], f32, name="tp", tag="tp")
                        nc.tensor.transpose(
                            tp, qr_sb[:, h * D : (h + 1) * D], ident
                        )
                        nc.vector.tensor_copy(qT_sb[:, h * BS : (h + 1) * BS], tp)
                    norm_rope(k_ps, kr_sb, 1, False)
                    tpk = psT.tile([128, BS], f32, name="tpk", tag="tp")
                    nc.tensor.transpose(tpk, kr_sb, ident)
                    nc.vector.tensor_copy(kT8, tpk)

            # Wo reuses Wq's SBUF slot. Its DMA is issued mid-stream on
            # the ACT ring (see group loop): by then phase A has long
            # freed wq, so the WAR wait is already satisfied and can't
            # head-of-line block the KV stream on the SP ring nor the
            # exp activations behind it on the ACT queue. Host
            # pre-interleaves Wo as [chunk][head][OCH] so each phase-C
            # chunk is one contiguous slab.
            wo_sb = sb.tile([128, G * HID], bf16, name="wo_sb", tag="bigw")

            qT3 = qT_sb.rearrange("p (h c) -> p h c", c=BS)
            attn3 = attn_T.rearrange("p (h c) -> p h c", c=BS)

            with tc.tile_pool(name="psB", bufs=1, space="PSUM") as psB:
                # K/V stream lookahead: kt DMAs run 4 groups ahead and vt
                # 2 ahead of compute, so the SP FIFO's buffer-WAR waits
                # (kt waits on S(g-4) done — an EARLY event; vt on
                # PV(g-4) — LATE) never let the PV pace starve the K
                # stream that gates the S matmuls. Patches are issued one
                # group ahead so their SWDGE/DVE latency hides under the
                # previous group's compute.
                ktt, vtt = {}, {}
                gsz = [sum(nb[b] for b in seqs) for _, seqs in groups]

                def ensure_kt(g):
                    if g >= len(groups) or g in ktt:
                        return
                    g0 = groups[g][0]
                    kt = sb.tile([128, gsz[g] * BLOCK_SIZE], f8e3,
                                 name=f"kt{g}", tag="kt", bufs=4)
                    nc.sync.dma_start(
                        out=kt,
                        in_=kp[:, g0 * BLOCK_SIZE : (g0 + gsz[g]) * BLOCK_SIZE],
                    )
                    ktt[g] = kt

                def ensure_vt(g):
                    if g >= len(groups) or g in vtt:
                        return
                    g0 = groups[g][0]
                    vt = sb.tile([128, gsz[g] * VW], bf16,
                                 name=f"vt{g}", tag="vt", bufs=4)
                    nc.sync.dma_start(
                        out=vt, in_=vp[:, g0 * VW : (g0 + gsz[g]) * VW]
                    )
                    vtt[g] = vt

                def patches(g):
                    # new token's k/v replace the seq's stale cache slot
                    vn3g = vtt[g].rearrange("p (n v) -> p n v", v=VW)
                    lb = 0
                    for b in groups[g][1]:
                        T = int(seq_lens[b])
                        r = (T - 1) % BLOCK_SIZE
                        le = lb + nb[b] - 1  # seq's last block, local idx
                        nc.vector.tensor_copy(
                            ktt[g][:, le * BLOCK_SIZE + r :
                                   le * BLOCK_SIZE + r + 1],
                            kT8[:, b : b + 1],
                        )
                        nc.gpsimd.dma_start(
                            out=vn3g[r : r + 1, le, 0:D], in_=vbf[b : b + 1, :]
                        )
                        lb += nb[b]

                # consumption-ordered stream: kt(g) then vt(g) per group,
                # two groups of lookahead — deep enough to hide trigger
                # latency, shallow enough that every DMA's buffer-WAR
                # wait is already satisfied when it reaches the FIFO head
                ensure_kt(0)
                ensure_vt(0)
                ensure_kt(1)
                ensure_vt(1)
                patches(0)

                # seqs pending normalize: deferred TWO seqs behind the
                # PV matmuls so the tp2 PE-transpose never head-of-line
                # blocks the PE queue on the ACT/DVE combine chain
                pending = []

                def normalize(b, ot_all, ns):
                    if ns == 1:
                        acc = ot_all[0:4, 0:VW]
                    else:
                        accs = smalls.tile([4, VW], f32, name=f"acc{b}",
                                           tag="acc")
                        nc.scalar.copy(accs, ot_all[0:4, 0:VW])
                        nc.vector.tensor_add(
                            accs, accs, ot_all[32:36, 512 : 512 + VW]
                        )
                        acc = accs
                    rec = smalls.tile([4, 1], f32, name=f"rec{b}", tag="rec")
                    nc.vector.reciprocal(rec, acc[:, D : D + 1])
                    o_sb = smalls.tile([4, D], f32, name=f"o{b}", tag="o")
                    nc.vector.tensor_scalar_mul(o_sb, acc[:, 0:D], rec)
                    # transposed o lands in this seq's ot buffer's spare
                    # columns (strip0 zero region, data already consumed)
                    tp2 = ot_all[:, 384:388]
                    nc.tensor.transpose(tp2, o_sb, ident[:4, :4])
                    nc.vector.tensor_copy(attn3[:, :, b], tp2)

                # S matmuls run one group AHEAD of exp/PV (group-level
                # software pipelining): while ACT runs exp(g), the PE
                # chews on S(g+1), so the PE queue never stalls at the
                # group-boundary exp/patch chain.
                stps = {}

                def emit_S(g):
                    if g >= len(groups):
                        return
                    nbg = gsz[g]
                    kt = ktt[g]
                    stp = psB.tile([128, 4 * nbg], f32, name=f"stp{g}",
                                   tag="stp", bufs=2)
                    lb = 0
                    for b in groups[g][1]:
                        qTb = qT3[:, :, b]
                        for j in range(nb[b]):
                            jl = lb + j
                            nc.tensor.matmul(
                                stp[:, 4 * jl : 4 * jl + 4],
                                kt[:, jl * BLOCK_SIZE : (jl + 1) * BLOCK_SIZE],
                                qTb,
                                start=True,
                                stop=True,
                            )
                        lb += nb[b]
                    stps[g] = stp

                emit_S(0)

                for gi, (g0, seqs) in enumerate(groups):
                    ensure_kt(gi + 2)
                    ensure_vt(gi + 2)
                    if gi + 1 < len(groups):
                        patches(gi + 1)
                        emit_S(gi + 1)

                    if gi == len(groups) - 2:
                        # wo goes on the SP ring HERE so its bytes queue
                        # BEHIND the kv stream in Q1's FIFO — on the ACT
                        # ring its transfer would start as soon as phase
                        # A frees the wq slot (~25us) and steal half the
                        # SDMA bandwidth from the kv stream. Its WAR wait
                        # (phase A) is long satisfied, so no FIFO stall.
                        for c in range(HID // OCH):
                            nc.sync.dma_start(
                                out=wo_sb[:, c * G * OCH : (c + 1) * G * OCH],
                                in_=wo[:, c * G * OCH : (c + 1) * G * OCH],
                            )
                    nbg = gsz[gi]
                    vn3 = vtt[gi].rearrange("p (n v) -> p n v", v=VW)
                    stp = stps.pop(gi)
                    e = sb.tile([128, 4 * nbg], bf16, name=f"e{gi}", tag="e", bufs=2)
                    # exp in sub-chunks so PV starts before all S mms finish
                    for c0 in range(0, nbg, EXPB):
                        c1 = min(c0 + EXPB, nbg)
                        nc.scalar.activation(
                            e[:, 4 * c0 : 4 * c1], stp[:, 4 * c0 : 4 * c1], AF.Exp
                        )

                    # PV, col-tiled: seq's blocks deal round-robin onto
                    # NSTRIP col-strips of the PE array; strips stream
                    # concurrently (own XBUS each), partials combined on
                    # ACT/DVE. Ones-column VW-1 carries the denominator.
                    # Each seq's normalize/transpose is DEFERRED until
                    # after the NEXT seq's PV matmuls (software
                    # pipelining): the tp2 PE-transpose then never
                    # head-of-line blocks the PE queue waiting on the
                    # DVE rec/mul chain.
                    lb = 0
                    for b in seqs:
                        nbb = nb[b]
                        # short seqs skip the strip split: the saved
                        # combine ops outweigh the lost PV concurrency
                        ns = 1 if nbb < 6 else min(NSTRIP, nbb)
                        # strip s of this seq accumulates at partitions
                        # 32s..32s+3 (matching tile_position col-group s)
                        # AND column offset 512s f32 — its own 2 KB PSUM
                        # zero region, so the interleaved strip groups
                        # can't interact (start=True invalidates per
                        # zero region); bufs=2 pipelines across seqs
                        ot_all = psB.tile([128, NSTRIP * 512], f32,
                                          name=f"ot{b}", tag="ot", bufs=3)
                        strip_blocks = [
                            [lb + j for j in range(s, nbb, ns)] for s in range(ns)
                        ]
                        maxlen = len(strip_blocks[0])
                        for r in range(maxlen):
                            for s in range(ns):
                                if r >= len(strip_blocks[s]):
                                    continue
                                jl = strip_blocks[s][r]
                                nc.tensor.matmul(
                                    ot_all[32 * s : 32 * s + 4,
                                           512 * s : 512 * s + VW],
                                    e[:, 4 * jl : 4 * jl + 4],
                                    vn3[:, jl, :],
                                    start=(r == 0),
                                    stop=(r == len(strip_blocks[s]) - 1),
                                    tile_position=(0, 32 * s),
                                )
                        lb += nbb
                        pending.append((b, ot_all, ns))
                        if len(pending) > 2:
                            normalize(*pending.pop(0))

                for p in pending:
                    normalize(*p)

            # phase C: heads (0,1) accumulate into PSUM strip 0 while
            # heads (2,3) run CONCURRENTLY in PE column-tile strip 1;
            # DVE copies strip 0 out and adds strip 1.
            ocs = sb.tile([BS, HID], bf16, name="ocs")
            with tc.tile_pool(name="psC", bufs=3, space="PSUM") as psC:
                for c in range(HID // 512):
                    oc = psC.tile([64, 512], f32, name=f"oc{c}", tag="oc")
                    for h in range(G):
                        nc.tensor.matmul(
                            oc[32 * (h // 2) : 32 * (h // 2) + 32, :],
                            attn_T[:, h * BS : (h + 1) * BS],
                            wo_sb[:, (c // 2) * G * OCH + h * OCH
                                  + (c % 2) * 512 :
                                  (c // 2) * G * OCH + h * OCH
                                  + (c % 2) * 512 + 512],
                            start=(h % 2 == 0), stop=(h % 2 == 1),
                            tile_position=(0, 32 * (h // 2)),
                        )
                    p01 = sb.tile([BS, 512], f32, name=f"p01_{c}", tag="p01", bufs=2)
                    nc.scalar.copy(p01, oc[0:32, :])
                    nc.vector.tensor_add(
                        ocs[:, c * 512 : (c + 1) * 512], p01, oc[32:64, :]
                    )
                    if c % 2 == 1:
                        # four out DMAs: earlier quarters overlap the
                        # later quarters' matmuls; only the last ~64 KB
                        # stays exposed at the end
                        nc.sync.dma_start(
                            out=outp[:, (c - 1) * 512 : (c + 1) * 512],
                            in_=ocs[:, (c - 1) * 512 : (c + 1) * 512],
                        )

    nc.compile()
    return nc


def _pack_w(w):
    # [4096, C] -> [128, KTILES*C]; sbuf[p, t*C + c] == w[t*128 + p, c]
    C = w.shape[1]
    return np.ascontiguousarray(
        w.reshape(KTILES, 128, C).transpose(1, 0, 2).reshape(128, KTILES * C)
    ).astype(BF16)


def _prepare(inputs):
    hs = np.ascontiguousarray(np.asarray(inputs["hidden_states"], np.float32)[0])
    Wq = np.asarray(inputs["Wq"], np.float32)
    Wk = np.asarray(inputs["Wk"], np.float32)
    Wv = np.asarray(inputs["Wv"], np.float32)
    Wo = np.asarray(inputs["Wo"], np.float32)
    cos_t = np.asarray(inputs["cos"], np.float32)[0]
    sin_t = np.asarray(inputs["sin"], np.float32)[0]
    qnw = np.asarray(inputs["q_norm_w"], np.float32)
    knw = np.asarray(inputs["k_norm_w"], np.float32)
    key_cache = np.asarray(inputs["key_cache"], np.float32)
    value_cache = np.asarray(inputs["value_cache"], np.float32)
    seq_lens = np.asarray(inputs["seq_lens_k"]).astype(np.int64)
    bt = np.asarray(inputs["block_table"]).astype(np.int64)

    apply_qw = not np.all(qnw == 1.0)
    apply_kw = not np.all(knw == 1.0)

    nb = [(int(t) + BLOCK_SIZE - 1) // BLOCK_SIZE for t in seq_lens]
    nbtot = sum(nb)
    order, groups = _plan(nb)
    blocks = np.concatenate([bt[b, : nb[b]] for b in order])

    hidT = np.ascontiguousarray(
        hs.T.reshape(KTILES, 128, BS).transpose(1, 0, 2).reshape(128, KTILES * BS)
    ).astype(BF16)

    # gather valid blocks once, cast once: K fp8 e3m4, V bf16
    kc_sel = key_cache[blocks].astype(F8E3)  # [nbtot, 128, HKV, D]
    vc_sel = value_cache[blocks].astype(BF16)

    # zero the invalid tail rows of each seq's last block (values AND
    # the ones/denominator column): garbage exp values for those rows
    # then multiply zeros on-device — no masking ops needed at all
    vmask = np.ones((128, nbtot), BF16)
    pos = 0
    for b in order:
        tmod = int(seq_lens[b]) % BLOCK_SIZE
        lastpos = pos + nb[b] - 1
        if tmod != 0:
            vc_sel[lastpos, tmod:, :, :] = 0
            vmask[tmod:, lastpos] = 0
        pos += nb[b]

    # first half of sin negated: rope becomes x*cos + xswap*sin_mod with
    # adds on both halves (see norm_rope)
    sin_mod = np.concatenate([-sin_t[:, : D // 2], sin_t[:, D // 2 :]], axis=1)
    cssn = np.ascontiguousarray(
        np.concatenate([cos_t, sin_mod, np.eye(BS, dtype=np.float32)], axis=1)
    )

    in_maps = []
    for i in range(NCORES):
        # K as [d, blk, tok] fp8; V as [tok, blk, d+1] bf16 with ones col D
        kp_i = np.ascontiguousarray(
            kc_sel[:, :, i, :].transpose(2, 0, 1).reshape(128, nbtot * BLOCK_SIZE)
        )
        v_all = vc_sel[:, :, i, :].transpose(1, 0, 2)  # [128 tok, nbtot, D]
        vp_i = np.empty((128, nbtot, VW), BF16)
        vp_i[:, :, 0:D] = v_all
        vp_i[:, :, D] = vmask
        vp_i = np.ascontiguousarray(vp_i.reshape(128, nbtot * VW))

        # Wo interleaved as [chunk][head][OCH] to match phase C's layout
        wo_i = (
            Wo[i * G * D : (i + 1) * G * D, :]
            .reshape(G, D, HID // OCH, OCH)
            .transpose(1, 2, 0, 3)
            .reshape(128, G * HID)
        )
        m = {
            "hidT": hidT,
            "wq": _pack_w(Wq[:, i * G * D : (i + 1) * G * D]),
            "wkv": _pack_w(
                np.concatenate(
                    [Wk[:, i * D : (i + 1) * D], Wv[:, i * D : (i + 1) * D]], axis=1
                )
            ),
            "wo": np.ascontiguousarray(wo_i).astype(BF16),
            "cssn": cssn,
            "kp": kp_i,
            "vp": vp_i,
        }
        if apply_qw:
            m["qw"] = np.ascontiguousarray(qnw.reshape(1, D))
        if apply_kw:
            m["kw"] = np.ascontiguousarray(knw.reshape(1, D))
        in_maps.append(m)

    key = (tuple(int(x) for x in seq_lens), apply_qw, apply_kw)
    if key not in _prog_cache:
        _prog_cache[key] = _build_program(seq_lens, apply_qw, apply_kw)
    nc = _prog_cache[key]
    return nc, in_maps


def kernel_with_stats(trace=False, **inputs):
    from concourse.bass_utils import run_bass_kernel_spmd

    nc, in_maps = _prepare(inputs)
    res = run_bass_kernel_spmd(
        nc, in_maps, core_ids=list(range(NCORES)), trace=trace
    )
    out = np.zeros((BS, HID), np.float32)
    for r in res.results:
        out += r["outp"].astype(np.float32)
    return out.reshape(1, BS, HID), res


def kernel(**inputs):
    out, _ = kernel_with_stats(trace=False, **inputs)
    return out
